# revision 13
# baseline (speedup 1.0000x reference)
"""Trainium2 Bass kernel for nn_Attention (dense transformer attention block).

Reference computation (per batch b):
  q = BN(wq @ x)  -> (8 heads, 16, 3136)
  k = BN(wk @ x)  -> (8, 16, 3136)
  v = BN(wv @ x)  -> (8, 64, 3136)
  attn = softmax(q^T k)  (scores over 3136x3136 tokens, no scaling)
  o = attn @ v^T -> (8, 64, 3136) -> (512, 56, 56)
  out = BN(wp @ o) -> (256, 56, 56)

Sharding: 8 cores = 2 batches x 4 token-chunks of 784 query tokens.
Each core computes k/v for the full 3136 tokens (cheap) and attention +
output projection for its own 784 query tokens. Zero collectives.

Device algorithm per core (flash-style, f32 end to end):
  - All BN scale factors folded into weights host-side; biases folded via an
    appended ones-row on x (K=257 contraction).
  - S_T[m, n-chunk] = k_blk^T q  (K=16), exp on ACT from PSUM,
    o'[65, n] += v'^T_blk @ exp(S_T_blk)  where v' has an appended ones
    column so row 64 of o' accumulates the softmax denominator.
  - o = o'[0:64] * reciprocal(o'[64]) broadcast via DMA.
  - out = wp_eff @ [o; 1].
"""

import os
import sys

for _p in ("/opt/trn_rl_repo", "/root/.axon_site/_ro/trn_rl_repo"):
    if os.path.isdir(_p) and _p not in sys.path:
        sys.path.insert(0, _p)

import numpy as np

NUM_HEADS = 8
KEY_DIM = 16
D_HEAD = 64
B = 2
C = 256
HH = 56
WW = 56
N = HH * WW          # 3136 tokens
NCHUNK = N // 4      # 784 query tokens per core
NSUB = NCHUNK // 2   # 392, fits one PSUM bank
NB = (N + 127) // 128            # 25 key-blocks
MB_SIZES = [128] * 24 + [64]
KS = [128, 128]                  # contraction chunks for K=256
GROUPS = [list(range(g * 3, min(g * 3 + 3, NB))) for g in range(9)]

_GRAPH = None


def _build_graph():
    import concourse.bass as bass  # noqa: F401
    import concourse.mybir as mybir
    import concourse.tile as tile
    from concourse import bacc
    from contextlib import ExitStack

    f32 = mybir.dt.float32
    bf16 = mybir.dt.bfloat16
    Exp = mybir.ActivationFunctionType.Exp

    nc = bacc.Bacc("TRN2", target_bir_lowering=False, debug=False, num_devices=8)
    xa_d = nc.dram_tensor("xa", [256, N], bf16, kind="ExternalInput").ap()
    xq_d = nc.dram_tensor("xq", [256, NCHUNK], bf16, kind="ExternalInput").ap()
    wq_d = nc.dram_tensor("wq", [256, 128], bf16, kind="ExternalInput").ap()
    wk_d = nc.dram_tensor("wk", [256, 128], bf16, kind="ExternalInput").ap()
    wv_d = nc.dram_tensor("wv", [256, 520], bf16, kind="ExternalInput").ap()
    qb_d = nc.dram_tensor("qb", [128, 1], f32, kind="ExternalInput").ap()
    kb_d = nc.dram_tensor("kb", [128, 1], f32, kind="ExternalInput").ap()
    vb_d = nc.dram_tensor("vb", [1, 520], bf16, kind="ExternalInput").ap()
    pb_d = nc.dram_tensor("pb", [128, 2], f32, kind="ExternalInput").ap()
    wp_d = nc.dram_tensor("wp", [64, 8, 256], bf16, kind="ExternalInput").ap()
    out_d = nc.dram_tensor("out", [256, NCHUNK], f32, kind="ExternalOutput").ap()
    rsd_d = nc.dram_tensor("rsd", [16, NSUB], f32).ap()  # rowsum bounce

    with tile.TileContext(nc) as tc, ExitStack() as stk:
        const = stk.enter_context(tc.tile_pool(name="const", bufs=1))
        xq_sb = const.tile([128, 2, NCHUNK], bf16, tag="xq")
        wq_sb = const.tile([128, 2, 128], bf16, tag="wq")
        wk_sb = const.tile([128, 2, 128], bf16, tag="wk")
        wv_sb = const.tile([128, 2, 520], bf16, tag="wv")
        wp_sb = const.tile([64, 8, 256], bf16, tag="wp")
        qb_sb = const.tile([128, 1], f32, tag="qb")
        kb_sb = const.tile([128, 1], f32, tag="kb")
        vb_sb = const.tile([128, 2, 260], bf16, tag="vb")
        pb_sb = const.tile([128, 2], f32, tag="pb")
        # per-head 32-aligned base partitions: head h -> (k_lo if h<4 else
        # k_hi) partitions [32*(h%4), 32*(h%4)+16)
        k_lo = const.tile([128, N], bf16, tag="klo")
        k_hi = const.tile([128, N], bf16, tag="khi")
        q_lo = const.tile([128, NCHUNK], bf16, tag="qlo")
        q_hi = const.tile([128, NCHUNK], bf16, tag="qhi")
        # replicas shifted by +32 partitions so consecutive blocks of one head
        # use different PE row groups (4-way concurrent scores)
        k_lo2 = const.tile([128, N], bf16, tag="klo2")
        k_hi2 = const.tile([128, N], bf16, tag="khi2")
        q_lo2 = const.tile([128, NCHUNK], bf16, tag="qlo2")
        q_hi2 = const.tile([128, NCHUNK], bf16, tag="qhi2")
        # v'^T: [m-in-block, block, head-half, 65*hh + (64 v cols + ones col)]
        vT_sb = const.tile([128, NB, 2, 260], bf16, tag="vt")
        of_sb = const.tile([64, 8, NCHUNK], bf16, tag="of")
        y_sb = const.tile([128, 2, NCHUNK], f32, tag="y")

        for kc in range(2):
            ks, off = KS[kc], 128 * kc
            nc.sync.dma_start(out=wq_sb[0:ks, kc, :], in_=wq_d[off:off + ks, :])
            nc.sync.dma_start(out=wk_sb[0:ks, kc, :], in_=wk_d[off:off + ks, :])
            nc.sync.dma_start(out=wv_sb[0:ks, kc, :], in_=wv_d[off:off + ks, :])
            nc.sync.dma_start(out=xq_sb[0:ks, kc, :], in_=xq_d[off:off + ks, :])
        nc.sync.dma_start(out=wp_sb[:], in_=wp_d[:])
        nc.sync.dma_start(out=qb_sb[:], in_=qb_d)
        nc.sync.dma_start(out=kb_sb[:], in_=kb_d)
        nc.sync.dma_start(out=pb_sb[:], in_=pb_d)
        nc.gpsimd.dma_start(out=vb_sb[:, :, :],
                            in_=vb_d.partition_broadcast(128))

        xa_sb = const.tile([128, 2, N], bf16, tag="xa")
        for kc in range(2):
            nc.sync.dma_start(out=xa_sb[:, kc, :],
                              in_=xa_d[128 * kc:128 * kc + 128, :])

        with tc.tile_pool(name="psA", bufs=2, space="PSUM") as psA, \
             tc.tile_pool(name="psAV", bufs=3, space="PSUM") as psAV, \
             tc.tile_pool(name="tmpA", bufs=1) as tA:
            k_sb = tA.tile([128, N], bf16, tag="ksb")
            q_sb = tA.tile([128, NCHUNK], bf16, tag="qsb")
            # q projection + immediate regroup
            for c2 in range(2):
                q_ps = psA.tile([128, 512], f32, tag="qkps")
                for kc in range(2):
                    nc.tensor.matmul(
                        q_ps[0:128, 0:NSUB],
                        wq_sb[0:KS[kc], kc, :],
                        xq_sb[0:KS[kc], kc, c2 * NSUB:(c2 + 1) * NSUB],
                        start=(kc == 0), stop=(kc == 1))
                nc.scalar.add(
                    q_sb[:, c2 * NSUB:(c2 + 1) * NSUB], q_ps[0:128, 0:NSUB],
                    qb_sb[:, 0:1])
            for h in range(8):
                qt = q_lo if h < 4 else q_hi
                qt2 = q_lo2 if h < 4 else q_hi2
                bp_ = 32 * (h % 4)
                bp2 = (bp_ + 32) % 128
                nc.sync.dma_start(out=qt[bp_:bp_ + 16, :], in_=q_sb[16 * h:16 * h + 16, :])
                nc.sync.dma_start(out=qt2[bp2:bp2 + 16, :], in_=q_sb[16 * h:16 * h + 16, :])
            # k projection, regrouped per 512-column pass
            for p in range(7):
                c0 = 512 * p
                cw = min(512, N - c0)
                k_ps = psA.tile([128, 512], f32, tag="qkps")
                for kc in range(2):
                    nc.tensor.matmul(
                        k_ps[0:128, 0:cw],
                        wk_sb[0:KS[kc], kc, :],
                        xa_sb[0:KS[kc], kc, c0:c0 + cw],
                        start=(kc == 0), stop=(kc == 1))
                nc.scalar.add(k_sb[:, c0:c0 + cw], k_ps[0:128, 0:cw],
                              kb_sb[:, 0:1])
                for h in range(8):
                    kt = k_lo if h < 4 else k_hi
                    kt2 = k_lo2 if h < 4 else k_hi2
                    bp_ = 32 * (h % 4)
                    bp2 = (bp_ + 32) % 128
                    nc.sync.dma_start(out=kt[bp_:bp_ + 16, c0:c0 + cw],
                                      in_=k_sb[16 * h:16 * h + 16, c0:c0 + cw])
                    nc.sync.dma_start(out=kt2[bp2:bp2 + 16, c0:c0 + cw],
                                      in_=k_sb[16 * h:16 * h + 16, c0:c0 + cw])
            # v'^T projection (runs last: the main loop's scores banks WAR
            # against these psum banks, so finish them as early as possible)
            for mb in range(NB):
                pb = MB_SIZES[mb]
                vt_ps = psAV.tile([128, 2, 512], f32, tag="vtps")
                for half in range(2):
                    for kc in range(2):
                        nc.tensor.matmul(
                            vt_ps[0:pb, half, 0:260],
                            xa_sb[0:KS[kc], kc, mb * 128:mb * 128 + pb],
                            wv_sb[0:KS[kc], kc, half * 260:(half + 1) * 260],
                            start=(kc == 0), stop=(kc == 1))
                nc.vector.tensor_add(
                    out=vT_sb[0:pb, mb, :, :], in0=vt_ps[0:pb, :, 0:260],
                    in1=vb_sb[0:pb, :, :])

        # main attention loop, software-pipelined:
        # iteration i = (head-pair, n-chunk). During iteration i's scores+exp
        # phase, the PE executes iteration i-1's o'-accumulation matmuls as
        # filler, so it never idles waiting on ACT (keeps HAM warm).
        # PSUM: scores 2 slots x 3 banks + o' 2 slots x 1 bank = 8 banks.
        PAIRS = [(0, 2), (1, 3), (4, 6), (5, 7)]
        ITERS = [(pair, c2) for pair in PAIRS for c2 in range(2)]

        def emit_scores_group(pair, c2, blocks, s_ps2, kts, qts, kts2, qts2,
                              bps, bps2):
            nc0 = c2 * NSUB
            for i, mb in enumerate(blocks):
                pbi = MB_SIZES[mb]
                for e in range(2):
                    if mb % 2 == 0:
                        kte, qte, be = kts[e], qts[e], bps[e]
                    else:
                        kte, qte, be = kts2[e], qts2[e], bps2[e]
                    nc.tensor.matmul(
                        s_ps2[e][0:pbi, i, 0:NSUB],
                        kte[be:be + 16, mb * 128:mb * 128 + pbi],
                        qte[be:be + 16, nc0:nc0 + NSUB],
                        start=True, stop=True,
                        tile_position=(be, 0))

        def emit_filler(job):
            # one o'-accumulation matmul of the previous iteration
            (pair, c2, e, p_tile, i, mb, o_ps2) = job
            h = pair[e]
            pbi = MB_SIZES[mb]
            nc.tensor.matmul(
                o_ps2[e][0:65, 0:NSUB],
                vT_sb[0:pbi, mb, h // 4, 65 * (h % 4):65 * (h % 4) + 65],
                p_tile[0:pbi, i, :],
                start=(mb == 0), stop=(mb == NB - 1))

        def emit_epilogue(pair, c2, o_ps2):
            nc0 = c2 * NSUB
            for e in range(2):
                h = pair[e]
                o_ps = o_ps2[e]
                idx = h * 2 + c2
                rsh = pEp.tile([128, NSUB], f32, tag="rsh")
                nc.vector.tensor_copy(rsh[64:65, :], o_ps[64:65, 0:NSUB])
                nc.sync.dma_start(out=rsd_d[idx:idx + 1, :], in_=rsh[64:65, :])
                rb = pEp.tile([64, NSUB], f32, tag="rb")
                nc.sync.dma_start(
                    out=rb[0:64, :],
                    in_=rsd_d[idx:idx + 1, :].partition_broadcast(64))
                rbr = pEp.tile([64, NSUB], f32, tag="rbr")
                scr = pEp.tile([64, NSUB], f32, tag="scr")
                nc.vector.reciprocal_approx_accurate(
                    out=rbr[:], in_=rb[0:64, :], scratch=scr[:])
                nc.vector.tensor_mul(
                    out=of_sb[0:64, h, nc0:nc0 + NSUB],
                    in0=o_ps[0:64, 0:NSUB], in1=rbr[:])

        with tc.tile_pool(name="pP", bufs=22) as pP, \
             tc.tile_pool(name="pEp", bufs=3) as pEp, \
             tc.tile_pool(name="psO", bufs=2, space="PSUM") as psO, \
             tc.tile_pool(name="psS", bufs=2, space="PSUM") as psS:
            prev = None  # (pair, c2, p_tiles) of the previous iteration
            for it in range(len(ITERS) + 1):
                cur = ITERS[it] if it < len(ITERS) else None
                fillers = []
                if prev is not None:
                    ppair, pc2, p_tiles = prev
                    o_ps2 = [psO.tile([128, 512], f32, tag="ops",
                                      name=f"ops{e}") for e in range(2)]
                    for mb in range(NB):
                        g, i = mb // 3, mb % 3
                        for e in range(2):
                            fillers.append((ppair, pc2, e, p_tiles[g][e],
                                            i, mb, o_ps2))
                if cur is None:
                    for job in fillers:
                        emit_filler(job)
                    emit_epilogue(ppair, pc2, o_ps2)
                    break
                pair, c2 = cur
                kts = [k_lo if h < 4 else k_hi for h in pair]
                qts = [q_lo if h < 4 else q_hi for h in pair]
                kts2 = [k_lo2 if h < 4 else k_hi2 for h in pair]
                qts2 = [q_lo2 if h < 4 else q_hi2 for h in pair]
                bps = [32 * (h % 4) for h in pair]
                bps2 = [(32 * (h % 4) + 32) % 128 for h in pair]
                p_tiles = []
                nfill = len(fillers)
                for g, blocks in enumerate(GROUPS):
                    gsz = len(blocks)
                    pb = MB_SIZES[blocks[-1]]
                    s_ps2 = [psS.tile([128, 3, 512], f32, tag="sps",
                                      name=f"sps{e}") for e in range(2)]
                    emit_scores_group(pair, c2, blocks, s_ps2, kts, qts,
                                      kts2, qts2, bps, bps2)
                    p_sb2 = [pP.tile([128, 3, NSUB], bf16, tag="psb",
                                     name=f"psb{e}") for e in range(2)]
                    for e in range(2):
                        nc.scalar.activation(
                            out=p_sb2[e][0:pb, 0:gsz, :],
                            in_=s_ps2[e][0:pb, 0:gsz, 0:NSUB], func=Exp)
                    p_tiles.append(p_sb2)
                    # interleave previous iteration's o' matmuls as PE filler
                    lo = nfill * g // len(GROUPS)
                    hi = nfill * (g + 1) // len(GROUPS)
                    for job in fillers[lo:hi]:
                        emit_filler(job)
                if prev is not None:
                    emit_epilogue(ppair, pc2, o_ps2)
                prev = (pair, c2, p_tiles)

        # output projection: out = wp_eff @ [o; 1]
        with tc.tile_pool(name="psY", bufs=2, space="PSUM") as psY:
            for mo in range(2):
                for c2 in range(2):
                    nc0 = c2 * NSUB
                    y_ps = psY.tile([128, 512], f32, tag="yps")
                    for kc in range(8):
                        nc.tensor.matmul(
                            y_ps[0:128, 0:NSUB],
                            wp_sb[0:64, kc, mo * 128:(mo + 1) * 128],
                            of_sb[0:64, kc, nc0:nc0 + NSUB],
                            start=(kc == 0), stop=(kc == 7))
                    nc.vector.tensor_scalar_add(
                        y_sb[:, mo, nc0:nc0 + NSUB], y_ps[0:128, 0:NSUB],
                        pb_sb[:, mo:mo + 1])
            for mo in range(2):
                nc.sync.dma_start(
                    out=out_d[mo * 128:(mo + 1) * 128, :], in_=y_sb[:, mo, :])

    nc.compile()
    return nc


def get_graph():
    global _GRAPH
    if _GRAPH is None:
        _GRAPH = _build_graph()
    return _GRAPH


def make_in_maps(x, wq, sq, bq, wk, sk, bk, wv, sv, bv, wp, sp, bp):
    import ml_dtypes
    bf = ml_dtypes.bfloat16
    f = np.float32
    x2 = np.asarray(x, f).reshape(B, C, N)
    ones_row = np.ones((1, N), f)
    wq = np.asarray(wq, f); sq = np.asarray(sq, f); bq = np.asarray(bq, f)
    wk = np.asarray(wk, f); sk = np.asarray(sk, f); bk = np.asarray(bk, f)
    wv = np.asarray(wv, f); sv = np.asarray(sv, f); bv = np.asarray(bv, f)
    wp = np.asarray(wp, f); sp = np.asarray(sp, f); bp = np.asarray(bp, f)

    wq_eff = (wq * sq[:, None]).T.astype(f)           # (256, 128)
    wk_eff = (wk * sk[:, None]).T.astype(f)
    wv_base = wv * sv[:, None]  # (512, 256)
    wv_arr = np.zeros((256, 520), f)
    vb_arr = np.zeros((1, 520), f)
    for h in range(NUM_HEADS):
        col = 260 * (h // 4) + 65 * (h % 4)
        wv_arr[:, col:col + 64] = wv_base[64 * h:64 * h + 64, :].T
        vb_arr[0, col:col + 64] = bv[64 * h:64 * h + 64]
        vb_arr[0, col + 64] = 1.0
    wp_eff = (wp * sp[:, None]).T.astype(f)  # (512, 256), row c = 64h+d
    wp_arr = wp_eff.reshape(8, 64, 256).transpose(1, 0, 2).copy()
    pb_arr = bp.reshape(2, 128).T.copy()  # (128, 2): pb_arr[d, mo] = bp[128*mo+d]
    in_maps = []
    for core in range(8):
        b, j = core // 4, core % 4
        xa_full = np.ascontiguousarray(x2[b])
        xq_c = np.ascontiguousarray(xa_full[:, j * NCHUNK:(j + 1) * NCHUNK])
        in_maps.append(dict(
            xa=xa_full.astype(bf), xq=xq_c.astype(bf),
            wq=wq_eff.astype(bf), wk=wk_eff.astype(bf),
            wv=wv_arr.astype(bf), wp=wp_arr.astype(bf),
            qb=bq.reshape(128, 1).astype(f), kb=bk.reshape(128, 1).astype(f),
            vb=vb_arr.astype(bf), pb=pb_arr.astype(f)))
    return in_maps


def assemble_output(results):
    y = np.zeros((B, C, N), np.float32)
    for core in range(8):
        b, j = core // 4, core % 4
        y[b, :, j * NCHUNK:(j + 1) * NCHUNK] = results[core]["out"]
    return y.reshape(B, C, HH, WW)


def kernel(**inputs):
    from concourse.bass_utils import run_bass_kernel_spmd
    nc = get_graph()
    in_maps = make_in_maps(**inputs)
    res = run_bass_kernel_spmd(nc, in_maps, core_ids=list(range(8)))
    return assemble_output(res.results)


if __name__ == "__main__":
    rng = np.random.default_rng(0)
    ins = dict(
        x=rng.standard_normal((2, 256, 56, 56), np.float32),
        wq=rng.standard_normal((128, 256), np.float32) * 0.05,
        sq=rng.random(128, np.float32),
        bq=rng.standard_normal(128, np.float32) * 0.05,
        wk=rng.standard_normal((128, 256), np.float32) * 0.05,
        sk=rng.random(128, np.float32),
        bk=rng.standard_normal(128, np.float32) * 0.05,
        wv=rng.standard_normal((512, 256), np.float32) * 0.05,
        sv=rng.random(512, np.float32),
        bv=rng.standard_normal(512, np.float32) * 0.05,
        wp=rng.standard_normal((256, 512), np.float32) * 0.05,
        sp=rng.random(256, np.float32),
        bp=rng.standard_normal(256, np.float32) * 0.05,
    )
    out = kernel(**ins)
    print("out", out.shape, out.dtype, float(np.abs(out).mean()))


# revision 14
# speedup vs baseline: 1.1901x; 1.1901x over previous
"""Trainium2 Bass kernel for nn_Attention (dense transformer attention block).

Reference computation (per batch b):
  q = BN(wq @ x)  -> (8 heads, 16, 3136)
  k = BN(wk @ x)  -> (8, 16, 3136)
  v = BN(wv @ x)  -> (8, 64, 3136)
  attn = softmax(q^T k)  (scores over 3136x3136 tokens, no scaling)
  o = attn @ v^T -> (8, 64, 3136) -> (512, 56, 56)
  out = BN(wp @ o) -> (256, 56, 56)

Sharding: 8 cores = 2 batches x 4 token-chunks of 784 query tokens.
Each core computes k/v for the full 3136 tokens (cheap) and attention +
output projection for its own 784 query tokens. Zero collectives.

Device algorithm per core (flash-style, f32 end to end):
  - All BN scale factors folded into weights host-side; biases folded via an
    appended ones-row on x (K=257 contraction).
  - S_T[m, n-chunk] = k_blk^T q  (K=16), exp on ACT from PSUM,
    o'[65, n] += v'^T_blk @ exp(S_T_blk)  where v' has an appended ones
    column so row 64 of o' accumulates the softmax denominator.
  - o = o'[0:64] * reciprocal(o'[64]) broadcast via DMA.
  - out = wp_eff @ [o; 1].
"""

import os
import sys

for _p in ("/opt/trn_rl_repo", "/root/.axon_site/_ro/trn_rl_repo"):
    if os.path.isdir(_p) and _p not in sys.path:
        sys.path.insert(0, _p)

import numpy as np

NUM_HEADS = 8
KEY_DIM = 16
D_HEAD = 64
B = 2
C = 256
HH = 56
WW = 56
N = HH * WW          # 3136 tokens
NCHUNK = N // 4      # 784 query tokens per core
NSUB = NCHUNK // 2   # 392, fits one PSUM bank
NB = (N + 127) // 128            # 25 key-blocks
MB_SIZES = [128] * 24 + [64]
KS = [128, 128]                  # contraction chunks for K=256
GROUPS = [list(range(g * 3, min(g * 3 + 3, NB))) for g in range(9)]

_GRAPH = None


def _build_graph():
    import concourse.bass as bass  # noqa: F401
    import concourse.mybir as mybir
    import concourse.tile as tile
    from concourse import bacc
    from contextlib import ExitStack

    f32 = mybir.dt.float32
    bf16 = mybir.dt.bfloat16
    Exp = mybir.ActivationFunctionType.Exp

    nc = bacc.Bacc("TRN2", target_bir_lowering=False, debug=False, num_devices=8)
    xa_d = nc.dram_tensor("xa", [256, N], bf16, kind="ExternalInput").ap()
    xq_d = nc.dram_tensor("xq", [256, NCHUNK], bf16, kind="ExternalInput").ap()
    wq_d = nc.dram_tensor("wq", [256, 128], bf16, kind="ExternalInput").ap()
    wk_d = nc.dram_tensor("wk", [256, 128], bf16, kind="ExternalInput").ap()
    wv_d = nc.dram_tensor("wv", [256, 520], bf16, kind="ExternalInput").ap()
    qb_d = nc.dram_tensor("qb", [128, 1], f32, kind="ExternalInput").ap()
    kb_d = nc.dram_tensor("kb", [128, 1], f32, kind="ExternalInput").ap()
    vb_d = nc.dram_tensor("vb", [1, 520], bf16, kind="ExternalInput").ap()
    pb_d = nc.dram_tensor("pb", [128, 2], f32, kind="ExternalInput").ap()
    wp_d = nc.dram_tensor("wp", [64, 8, 256], bf16, kind="ExternalInput").ap()
    out_d = nc.dram_tensor("out", [256, NCHUNK], f32, kind="ExternalOutput").ap()
    rsd_d = nc.dram_tensor("rsd", [16, NSUB], f32).ap()  # rowsum bounce

    with tile.TileContext(nc) as tc, ExitStack() as stk:
        const = stk.enter_context(tc.tile_pool(name="const", bufs=1))
        xq_sb = const.tile([128, 2, NCHUNK], bf16, tag="xq")
        wq_sb = const.tile([128, 2, 128], bf16, tag="wq")
        wk_sb = const.tile([128, 2, 128], bf16, tag="wk")
        wv_sb = const.tile([128, 2, 520], bf16, tag="wv")
        wp_sb = const.tile([64, 8, 256], bf16, tag="wp")
        qb_sb = const.tile([128, 1], f32, tag="qb")
        kb_sb = const.tile([128, 1], f32, tag="kb")
        vb_sb = const.tile([128, 2, 260], bf16, tag="vb")
        pb_sb = const.tile([128, 2], f32, tag="pb")
        # per-head 32-aligned base partitions: head h -> (k_lo if h<4 else
        # k_hi) partitions [32*(h%4), 32*(h%4)+16)
        k_lo = const.tile([128, N], bf16, tag="klo")
        k_hi = const.tile([128, N], bf16, tag="khi")
        q_lo = const.tile([128, NCHUNK], bf16, tag="qlo")
        q_hi = const.tile([128, NCHUNK], bf16, tag="qhi")
        # replicas shifted by +32 partitions so consecutive blocks of one head
        # use different PE row groups (4-way concurrent scores)
        k_lo2 = const.tile([128, N], bf16, tag="klo2")
        k_hi2 = const.tile([128, N], bf16, tag="khi2")
        q_lo2 = const.tile([128, NCHUNK], bf16, tag="qlo2")
        q_hi2 = const.tile([128, NCHUNK], bf16, tag="qhi2")
        # v'^T: [m-in-block, block, head-half, 65*hh + (64 v cols + ones col)]
        vT_sb = const.tile([128, NB, 2, 260], bf16, tag="vt")
        of_sb = const.tile([64, 8, NCHUNK], bf16, tag="of")
        y_sb = const.tile([128, 2, NCHUNK], f32, tag="y")

        for kc in range(2):
            ks, off = KS[kc], 128 * kc
            nc.sync.dma_start(out=wq_sb[0:ks, kc, :], in_=wq_d[off:off + ks, :])
            nc.sync.dma_start(out=wk_sb[0:ks, kc, :], in_=wk_d[off:off + ks, :])
            nc.sync.dma_start(out=wv_sb[0:ks, kc, :], in_=wv_d[off:off + ks, :])
            nc.sync.dma_start(out=xq_sb[0:ks, kc, :], in_=xq_d[off:off + ks, :])
        nc.sync.dma_start(out=wp_sb[:], in_=wp_d[:])
        nc.sync.dma_start(out=qb_sb[:], in_=qb_d)
        nc.sync.dma_start(out=kb_sb[:], in_=kb_d)
        nc.sync.dma_start(out=pb_sb[:], in_=pb_d)
        nc.gpsimd.dma_start(out=vb_sb[:, :, :],
                            in_=vb_d.partition_broadcast(128))

        xa_sb = const.tile([128, 2, N], bf16, tag="xa")
        for kc in range(2):
            nc.sync.dma_start(out=xa_sb[:, kc, :],
                              in_=xa_d[128 * kc:128 * kc + 128, :])

        with tc.tile_pool(name="psA", bufs=2, space="PSUM") as psA, \
             tc.tile_pool(name="psAV", bufs=3, space="PSUM") as psAV, \
             tc.tile_pool(name="tmpA", bufs=1) as tA:
            k_sb = tA.tile([128, N], bf16, tag="ksb")
            q_sb = tA.tile([128, NCHUNK], bf16, tag="qsb")
            # q projection + immediate regroup
            for c2 in range(2):
                q_ps = psA.tile([128, 512], f32, tag="qkps")
                for kc in range(2):
                    nc.tensor.matmul(
                        q_ps[0:128, 0:NSUB],
                        wq_sb[0:KS[kc], kc, :],
                        xq_sb[0:KS[kc], kc, c2 * NSUB:(c2 + 1) * NSUB],
                        start=(kc == 0), stop=(kc == 1))
                nc.scalar.add(
                    q_sb[:, c2 * NSUB:(c2 + 1) * NSUB], q_ps[0:128, 0:NSUB],
                    qb_sb[:, 0:1])
            for h in range(8):
                qt = q_lo if h < 4 else q_hi
                qt2 = q_lo2 if h < 4 else q_hi2
                bp_ = 32 * (h % 4)
                bp2 = (bp_ + 32) % 128
                nc.gpsimd.dma_start(out=qt[bp_:bp_ + 16, :], in_=q_sb[16 * h:16 * h + 16, :])
                nc.gpsimd.dma_start(out=qt2[bp2:bp2 + 16, :], in_=q_sb[16 * h:16 * h + 16, :])
            # k projection, regrouped per 512-column pass
            for p in range(7):
                c0 = 512 * p
                cw = min(512, N - c0)
                k_ps = psA.tile([128, 512], f32, tag="qkps")
                for kc in range(2):
                    nc.tensor.matmul(
                        k_ps[0:128, 0:cw],
                        wk_sb[0:KS[kc], kc, :],
                        xa_sb[0:KS[kc], kc, c0:c0 + cw],
                        start=(kc == 0), stop=(kc == 1))
                nc.scalar.add(k_sb[:, c0:c0 + cw], k_ps[0:128, 0:cw],
                              kb_sb[:, 0:1])
            for h in range(8):
                kt = k_lo if h < 4 else k_hi
                kt2 = k_lo2 if h < 4 else k_hi2
                bp_ = 32 * (h % 4)
                bp2 = (bp_ + 32) % 128
                nc.gpsimd.dma_start(out=kt[bp_:bp_ + 16, :],
                                    in_=k_sb[16 * h:16 * h + 16, :])
                nc.gpsimd.dma_start(out=kt2[bp2:bp2 + 16, :],
                                    in_=k_sb[16 * h:16 * h + 16, :])
            # v'^T projection (runs last: the main loop's scores banks WAR
            # against these psum banks, so finish them as early as possible)
            for mb in range(NB):
                pb = MB_SIZES[mb]
                vt_ps = psAV.tile([128, 2, 512], f32, tag="vtps")
                for half in range(2):
                    for kc in range(2):
                        nc.tensor.matmul(
                            vt_ps[0:pb, half, 0:260],
                            xa_sb[0:KS[kc], kc, mb * 128:mb * 128 + pb],
                            wv_sb[0:KS[kc], kc, half * 260:(half + 1) * 260],
                            start=(kc == 0), stop=(kc == 1))
                nc.vector.tensor_add(
                    out=vT_sb[0:pb, mb, :, :], in0=vt_ps[0:pb, :, 0:260],
                    in1=vb_sb[0:pb, :, :])

        # main attention loop, software-pipelined:
        # iteration i = (head-pair, n-chunk). During iteration i's scores+exp
        # phase, the PE executes iteration i-1's o'-accumulation matmuls as
        # filler, so it never idles waiting on ACT (keeps HAM warm).
        # PSUM: scores 2 slots x 3 banks + o' 2 slots x 1 bank = 8 banks.
        PAIRS = [(0, 2), (1, 3), (4, 6), (5, 7)]
        ITERS = [(pair, c2) for pair in PAIRS for c2 in range(2)]

        def emit_scores_group(pair, c2, blocks, s_ps2, kts, qts, kts2, qts2,
                              bps, bps2):
            nc0 = c2 * NSUB
            for i, mb in enumerate(blocks):
                pbi = MB_SIZES[mb]
                for e in range(2):
                    if mb % 2 == 0:
                        kte, qte, be = kts[e], qts[e], bps[e]
                    else:
                        kte, qte, be = kts2[e], qts2[e], bps2[e]
                    nc.tensor.matmul(
                        s_ps2[e][0:pbi, i, 0:NSUB],
                        kte[be:be + 16, mb * 128:mb * 128 + pbi],
                        qte[be:be + 16, nc0:nc0 + NSUB],
                        start=True, stop=True,
                        tile_position=(be, 0))

        def emit_filler(job):
            # one o'-accumulation matmul of the previous iteration
            (pair, c2, e, p_tile, i, mb, o_ps2) = job
            h = pair[e]
            pbi = MB_SIZES[mb]
            nc.tensor.matmul(
                o_ps2[e][0:65, 0:NSUB],
                vT_sb[0:pbi, mb, h // 4, 65 * (h % 4):65 * (h % 4) + 65],
                p_tile[0:pbi, i, :],
                start=(mb == 0), stop=(mb == NB - 1))

        def emit_epilogue(pair, c2, o_ps2):
            nc0 = c2 * NSUB
            for e in range(2):
                h = pair[e]
                o_ps = o_ps2[e]
                idx = h * 2 + c2
                rsh = pEp.tile([128, NSUB], f32, tag="rsh")
                nc.vector.tensor_copy(rsh[64:65, :], o_ps[64:65, 0:NSUB])
                nc.sync.dma_start(out=rsd_d[idx:idx + 1, :], in_=rsh[64:65, :])
                rb = pEp.tile([64, NSUB], f32, tag="rb")
                nc.sync.dma_start(
                    out=rb[0:64, :],
                    in_=rsd_d[idx:idx + 1, :].partition_broadcast(64))
                rbr = pEp.tile([64, NSUB], f32, tag="rbr")
                scr = pEp.tile([64, NSUB], f32, tag="scr")
                nc.vector.reciprocal_approx_accurate(
                    out=rbr[:], in_=rb[0:64, :], scratch=scr[:])
                nc.vector.tensor_mul(
                    out=of_sb[0:64, h, nc0:nc0 + NSUB],
                    in0=o_ps[0:64, 0:NSUB], in1=rbr[:])

        with tc.tile_pool(name="pP", bufs=22) as pP, \
             tc.tile_pool(name="pEp", bufs=3) as pEp, \
             tc.tile_pool(name="psO", bufs=2, space="PSUM") as psO, \
             tc.tile_pool(name="psS", bufs=2, space="PSUM") as psS:
            prev = None  # (pair, c2, p_tiles) of the previous iteration
            for it in range(len(ITERS) + 1):
                cur = ITERS[it] if it < len(ITERS) else None
                fillers = []
                if prev is not None:
                    ppair, pc2, p_tiles = prev
                    o_ps2 = [psO.tile([128, 512], f32, tag="ops",
                                      name=f"ops{e}") for e in range(2)]
                    for mb in range(NB):
                        g, i = mb // 3, mb % 3
                        for e in range(2):
                            fillers.append((ppair, pc2, e, p_tiles[g][e],
                                            i, mb, o_ps2))
                if cur is None:
                    for job in fillers:
                        emit_filler(job)
                    emit_epilogue(ppair, pc2, o_ps2)
                    break
                pair, c2 = cur
                kts = [k_lo if h < 4 else k_hi for h in pair]
                qts = [q_lo if h < 4 else q_hi for h in pair]
                kts2 = [k_lo2 if h < 4 else k_hi2 for h in pair]
                qts2 = [q_lo2 if h < 4 else q_hi2 for h in pair]
                bps = [32 * (h % 4) for h in pair]
                bps2 = [(32 * (h % 4) + 32) % 128 for h in pair]
                p_tiles = []
                nfill = len(fillers)
                for g, blocks in enumerate(GROUPS):
                    gsz = len(blocks)
                    pb = MB_SIZES[blocks[-1]]
                    s_ps2 = [psS.tile([128, 3, 512], f32, tag="sps",
                                      name=f"sps{e}") for e in range(2)]
                    emit_scores_group(pair, c2, blocks, s_ps2, kts, qts,
                                      kts2, qts2, bps, bps2)
                    p_sb2 = [pP.tile([128, 3, NSUB], bf16, tag="psb",
                                     name=f"psb{e}") for e in range(2)]
                    for e in range(2):
                        nc.scalar.activation(
                            out=p_sb2[e][0:pb, 0:gsz, :],
                            in_=s_ps2[e][0:pb, 0:gsz, 0:NSUB], func=Exp)
                    p_tiles.append(p_sb2)
                    # interleave previous iteration's o' matmuls as PE filler
                    lo = nfill * g // len(GROUPS)
                    hi = nfill * (g + 1) // len(GROUPS)
                    for job in fillers[lo:hi]:
                        emit_filler(job)
                if prev is not None:
                    emit_epilogue(ppair, pc2, o_ps2)
                prev = (pair, c2, p_tiles)

        # output projection: out = wp_eff @ [o; 1]
        with tc.tile_pool(name="psY", bufs=2, space="PSUM") as psY:
            for mo in range(2):
                for c2 in range(2):
                    nc0 = c2 * NSUB
                    y_ps = psY.tile([128, 512], f32, tag="yps")
                    for kc in range(8):
                        nc.tensor.matmul(
                            y_ps[0:128, 0:NSUB],
                            wp_sb[0:64, kc, mo * 128:(mo + 1) * 128],
                            of_sb[0:64, kc, nc0:nc0 + NSUB],
                            start=(kc == 0), stop=(kc == 7))
                    nc.vector.tensor_scalar_add(
                        y_sb[:, mo, nc0:nc0 + NSUB], y_ps[0:128, 0:NSUB],
                        pb_sb[:, mo:mo + 1])
            for mo in range(2):
                nc.sync.dma_start(
                    out=out_d[mo * 128:(mo + 1) * 128, :], in_=y_sb[:, mo, :])

    nc.compile()
    return nc


def get_graph():
    global _GRAPH
    if _GRAPH is None:
        _GRAPH = _build_graph()
    return _GRAPH


def make_in_maps(x, wq, sq, bq, wk, sk, bk, wv, sv, bv, wp, sp, bp):
    import ml_dtypes
    bf = ml_dtypes.bfloat16
    f = np.float32
    x2 = np.asarray(x, f).reshape(B, C, N)
    ones_row = np.ones((1, N), f)
    wq = np.asarray(wq, f); sq = np.asarray(sq, f); bq = np.asarray(bq, f)
    wk = np.asarray(wk, f); sk = np.asarray(sk, f); bk = np.asarray(bk, f)
    wv = np.asarray(wv, f); sv = np.asarray(sv, f); bv = np.asarray(bv, f)
    wp = np.asarray(wp, f); sp = np.asarray(sp, f); bp = np.asarray(bp, f)

    wq_eff = (wq * sq[:, None]).T.astype(f)           # (256, 128)
    wk_eff = (wk * sk[:, None]).T.astype(f)
    wv_base = wv * sv[:, None]  # (512, 256)
    wv_arr = np.zeros((256, 520), f)
    vb_arr = np.zeros((1, 520), f)
    for h in range(NUM_HEADS):
        col = 260 * (h // 4) + 65 * (h % 4)
        wv_arr[:, col:col + 64] = wv_base[64 * h:64 * h + 64, :].T
        vb_arr[0, col:col + 64] = bv[64 * h:64 * h + 64]
        vb_arr[0, col + 64] = 1.0
    wp_eff = (wp * sp[:, None]).T.astype(f)  # (512, 256), row c = 64h+d
    wp_arr = wp_eff.reshape(8, 64, 256).transpose(1, 0, 2).copy()
    pb_arr = bp.reshape(2, 128).T.copy()  # (128, 2): pb_arr[d, mo] = bp[128*mo+d]
    in_maps = []
    for core in range(8):
        b, j = core // 4, core % 4
        xa_full = np.ascontiguousarray(x2[b])
        xq_c = np.ascontiguousarray(xa_full[:, j * NCHUNK:(j + 1) * NCHUNK])
        in_maps.append(dict(
            xa=xa_full.astype(bf), xq=xq_c.astype(bf),
            wq=wq_eff.astype(bf), wk=wk_eff.astype(bf),
            wv=wv_arr.astype(bf), wp=wp_arr.astype(bf),
            qb=bq.reshape(128, 1).astype(f), kb=bk.reshape(128, 1).astype(f),
            vb=vb_arr.astype(bf), pb=pb_arr.astype(f)))
    return in_maps


def assemble_output(results):
    y = np.zeros((B, C, N), np.float32)
    for core in range(8):
        b, j = core // 4, core % 4
        y[b, :, j * NCHUNK:(j + 1) * NCHUNK] = results[core]["out"]
    return y.reshape(B, C, HH, WW)


def kernel(**inputs):
    from concourse.bass_utils import run_bass_kernel_spmd
    nc = get_graph()
    in_maps = make_in_maps(**inputs)
    res = run_bass_kernel_spmd(nc, in_maps, core_ids=list(range(8)))
    return assemble_output(res.results)


if __name__ == "__main__":
    rng = np.random.default_rng(0)
    ins = dict(
        x=rng.standard_normal((2, 256, 56, 56), np.float32),
        wq=rng.standard_normal((128, 256), np.float32) * 0.05,
        sq=rng.random(128, np.float32),
        bq=rng.standard_normal(128, np.float32) * 0.05,
        wk=rng.standard_normal((128, 256), np.float32) * 0.05,
        sk=rng.random(128, np.float32),
        bk=rng.standard_normal(128, np.float32) * 0.05,
        wv=rng.standard_normal((512, 256), np.float32) * 0.05,
        sv=rng.random(512, np.float32),
        bv=rng.standard_normal(512, np.float32) * 0.05,
        wp=rng.standard_normal((256, 512), np.float32) * 0.05,
        sp=rng.random(256, np.float32),
        bp=rng.standard_normal(256, np.float32) * 0.05,
    )
    out = kernel(**ins)
    print("out", out.shape, out.dtype, float(np.abs(out).mean()))
